# revision 1
# baseline (speedup 1.0000x reference)
"""Trainium2 Bass kernel: Aaren-style online-softmax linear-attention scan.

Math (per (b,h) pair, per timestep t):
    alpha_t = q_t . k_t                       (scalar)
    y_t = sum_{s<=t} exp(alpha_s - C_t) v_s / sum_{s<=t} exp(alpha_s - C_t)
for any stabilizer C_t >= max (the ratio is invariant). We use a running
*chunk* max M_c, mathematically identical to the reference's running max.

Layout: the host pre-permutes q,k,v per (b,h) pair into
    x_perm[pair, p, c, d] = x[pair, c*127 + (p-1), d]   (p in 1..127)
with row p=0 zeroed (it carries the inter-chunk state on chip) and the tail
chunk zero-padded. Each device-side load/store is then one fully contiguous
[128, nch*128] DMA (16.5 KB/partition descriptors run at ~620 GB/s/core,
vs ~20 GB/s for the 512 B/partition descriptors of the natural layout).

Per chunk c one fp32 TensorE matmul with a constant triangular stationary
U2 (U2[s,t] = t>=s, col 0 forced to ones) computes every prefix numerator
and the end-of-chunk carry:
    psum[t, f] = sum_s U2[s, t] * w_s * mv[s, f]
where mv rows are [carry; v_1..v_127] scaled in place by w (w_0 = gamma_c =
exp(M_{c-1} - M_c) rebases the carry). psum row 0 = next carry, copied to
row 0 of the next chunk's v block.

Denominators for a whole pair come from two more matmuls into one PSUM
tile: cumw[t, c] (+gamma_c) from U2 @ W, then a rank-1 accumulate adds
gamma_c*(D_{c-1} - 1) per column, where D_c is a 33-step scalar recurrence
done with one DVE tensor_tensor_scan. One reciprocal per pair gives
1/d[t, c]; ScalarE applies it while evacuating PSUM (y = n * (1/d)).

Sharding: B*H = 64 pairs -> 8 pairs per NeuronCore, no cross-core traffic.
"""

import sys

if "/opt/trn_rl_repo" not in sys.path:
    sys.path.insert(0, "/opt/trn_rl_repo")

import numpy as np

B, H, N, D = 4, 16, 4096, 128
NCORES = 8
PAIRS = B * H // NCORES  # 8 (b,h) pairs per core

CH = 127           # timesteps per chunk (partition row 0 is the carry)
NCH = -(-N // CH)  # 33 chunks
FW = NCH * D       # free width of the packed per-pair tiles
NEG = -3.0e38


def build_nc(pairs=PAIRS, n=N, repeats=1, mode="full", group=3,
             scale_engine="gpsimd"):
    import concourse.tile as tile
    from concourse import bacc, mybir
    from contextlib import ExitStack

    do_dma = mode in ("full", "dma")
    do_cmp = mode in ("full", "compute")

    f32 = mybir.dt.float32
    Alu = mybir.AluOpType
    Act = mybir.ActivationFunctionType
    X = mybir.AxisListType.X

    nch = -(-n // CH)
    fw = nch * D

    nc = bacc.Bacc("TRN2", target_bir_lowering=False, debug=False)

    qd = nc.dram_tensor("q", [pairs, 128, fw], f32, kind="ExternalInput")
    kd = nc.dram_tensor("k", [pairs, 128, fw], f32, kind="ExternalInput")
    vd = nc.dram_tensor("v", [pairs, 128, fw], f32, kind="ExternalInput")
    yd = nc.dram_tensor("y", [pairs, 128, fw], f32, kind="ExternalOutput")

    with tile.TileContext(nc) as tc, ExitStack() as ctx:
        cpool = ctx.enter_context(tc.tile_pool(name="consts", bufs=1))
        apool = ctx.enter_context(tc.tile_pool(name="alpha", bufs=1))
        wpool = ctx.enter_context(tc.tile_pool(name="wts", bufs=1))
        rpool = ctx.enter_context(tc.tile_pool(name="rcps", bufs=1))
        spool = ctx.enter_context(tc.tile_pool(name="stats", bufs=2))
        qkpool = ctx.enter_context(tc.tile_pool(name="qk", bufs=2))
        prpool = ctx.enter_context(tc.tile_pool(name="prod", bufs=2))
        vpool = ctx.enter_context(tc.tile_pool(name="vv", bufs=group))
        ypool = ctx.enter_context(tc.tile_pool(name="yy", bufs=group))
        pspool = ctx.enter_context(tc.tile_pool(name="ps", bufs=1, space="PSUM"))

        scale_eng = {"gpsimd": nc.gpsimd, "vector": nc.vector}[scale_engine]

        # ---- constants -------------------------------------------------
        iota_f = cpool.tile([128, 128], f32, tag="iotaf")
        nc.gpsimd.iota(iota_f[:], [[1, 128]], channel_multiplier=0,
                       allow_small_or_imprecise_dtypes=True)
        iota_p = cpool.tile([128, 1], f32, tag="iotap")
        nc.gpsimd.iota(iota_p[:], [[0, 1]], channel_multiplier=1,
                       allow_small_or_imprecise_dtypes=True)
        u2 = cpool.tile([128, 128], f32, tag="u2")
        # u2[s, t] = 1.0 if t >= s else 0.0; col 0 = ones (carry output row)
        nc.vector.tensor_scalar(u2[:], iota_f[:], iota_p[:], None, Alu.is_ge)
        nc.gpsimd.memset(u2[:, 0:1], 1.0)
        ident = cpool.tile([128, 128], f32, tag="ident")
        nc.vector.tensor_scalar(ident[:], iota_f[:], iota_p[:], None,
                                Alu.is_equal)
        # triangular without the carry row: row 0 = 0 (col 0 = full sums)
        u2z = cpool.tile([128, 128], f32, tag="u2z")
        nc.vector.tensor_scalar(u2z[:], iota_f[:], iota_p[:], None, Alu.is_ge)
        nc.gpsimd.memset(u2z[:, 0:1], 1.0)
        nc.gpsimd.memset(u2z[0:1, :], 0.0)
        ones_row = cpool.tile([1, 128], f32, tag="onesrow")
        nc.gpsimd.memset(ones_row[:], 1.0)

        ps = [pspool.tile([128, 129], f32, tag=f"ps{p}", name=f"ps{p}")
              for p in range(pairs)]

        for _rep in range(repeats):
            A, W, R = [], [], []
            # ---- phase 1: alpha + weights + denominators, per pair -----
            for p in range(pairs):
                qall = qkpool.tile([128, fw], f32, tag="qa")
                kall = qkpool.tile([128, fw], f32, tag="ka")
                if do_dma:
                    nc.sync.dma_start(qall[:], qd[p])
                    nc.sync.dma_start(kall[:], kd[p])
                Ap = apool.tile([128, nch], f32, tag=f"A{p}")
                A.append(Ap)
                if not do_cmp:
                    continue
                prod = prpool.tile([128, fw], f32, tag="pr")
                nc.vector.tensor_mul(prod[:], qall[:], kall[:])
                nc.vector.tensor_reduce(
                    Ap[:], prod[:].rearrange("p (c d) -> p c d", d=D),
                    axis=X, op=Alu.add)

                # -- stabilizers: chunk maxes, running max, w, gamma -----
                psb = ps[p]
                nc.tensor.transpose(psb[0:nch, 0:128], Ap[:], ident[:])
                mu_sb = spool.tile([128, 1], f32, tag="mu")
                nc.vector.tensor_reduce(mu_sb[0:nch, :], psb[0:nch, 0:128],
                                        axis=X, op=Alu.max)
                nc.tensor.transpose(psb[0:1, 0:nch], mu_sb[0:nch, :],
                                    ident[0:nch, 0:nch])
                mrow = spool.tile([1, nch], f32, tag="mrow")
                nc.vector.tensor_copy(mrow[0:1, :], psb[0:1, 0:nch])
                Mrow = spool.tile([1, nch], f32, tag="Mrow")
                nc.vector.tensor_tensor_scan(Mrow[0:1, :], mrow[0:1, :],
                                             mrow[0:1, :], initial=NEG,
                                             op0=Alu.max, op1=Alu.max)
                # broadcast M to all partitions; W = exp(A - M)
                nc.tensor.matmul(psb[0:128, 0:nch], ones_row[0:1, :],
                                 Mrow[0:1, :], start=True, stop=True)
                Wp = wpool.tile([128, nch], f32, tag=f"W{p}")
                W.append(Wp)
                nc.vector.tensor_sub(Wp[:], Ap[:], psb[0:128, 0:nch])
                nc.scalar.activation(Wp[:], Wp[:], Act.Exp)
                # gamma row: gamma_c = exp(M_{c-1} - M_c), gamma_0 = 1
                g1 = spool.tile([1, nch], f32, tag="g1")
                if nch > 1:
                    nc.vector.tensor_copy(g1[0:1, 1:nch], Mrow[0:1, 0:nch - 1])
                nc.vector.tensor_copy(g1[0:1, 0:1], Mrow[0:1, 0:1])
                g2 = spool.tile([1, nch], f32, tag="g2")
                nc.vector.tensor_sub(g2[0:1, :], g1[0:1, :], Mrow[0:1, :])
                nc.scalar.activation(Wp[0:1, :], g2[0:1, :], Act.Exp)

                # -- denominators: d[t, c] for the whole pair ------------
                # column sums of W (= gamma_c + S_c) via transpose+reduce
                nc.tensor.transpose(psb[0:nch, 0:128], Wp[:], ident[:])
                sw = spool.tile([128, 1], f32, tag="sw")
                nc.vector.tensor_reduce(sw[0:nch, :], psb[0:nch, 0:128],
                                        axis=X, op=Alu.add)
                nc.tensor.transpose(psb[0:1, 0:nch], sw[0:nch, :],
                                    ident[0:nch, 0:nch])
                srow = spool.tile([1, nch], f32, tag="srow")
                nc.scalar.copy(srow[0:1, :], psb[0:1, 0:nch])
                s2 = spool.tile([1, nch], f32, tag="s2")
                nc.vector.tensor_sub(s2[0:1, :], srow[0:1, :], Wp[0:1, :])
                # D_c = gamma_c * D_{c-1} + S_c  (33-step scalar recurrence)
                Drow = spool.tile([1, nch], f32, tag="Drow")
                nc.vector.tensor_tensor_scan(Drow[0:1, :], Wp[0:1, :],
                                             s2[0:1, :], initial=0.0,
                                             op0=Alu.mult, op1=Alu.add)
                dp1 = spool.tile([1, nch], f32, tag="dp1")
                if nch > 1:
                    nc.vector.tensor_copy(dp1[0:1, 1:nch], Drow[0:1, 0:nch - 1])
                nc.gpsimd.memset(dp1[0:1, 0:1], 0.0)
                adj = spool.tile([1, nch], f32, tag="adj")
                nc.vector.tensor_mul(adj[0:1, :], Wp[0:1, :], dp1[0:1, :])
                # d[t, c] = cumw (no gamma term) + gamma_c * D_{c-1}
                dps = psb[0:128, 0:nch]
                nc.tensor.matmul(dps, u2z[:], Wp[:], start=True, stop=False)
                nc.tensor.matmul(dps, ones_row[0:1, :], adj[0:1, :],
                                 start=False, stop=True)
                Rp = rpool.tile([128, nch], f32, tag=f"R{p}")
                R.append(Rp)
                nc.vector.reciprocal(Rp[:], dps)

            # ---- phase 2: chunked triangular matmuls with carry chain --
            for g0 in range(0, pairs, group):
                gp = list(range(g0, min(g0 + group, pairs)))
                vall, yall = {}, {}
                for p in gp:
                    vall[p] = vpool.tile([128, fw], f32, tag="va",
                                         name=f"va{p}")
                    yall[p] = ypool.tile([128, fw], f32, tag="ya",
                                         name=f"ya{p}")
                    if do_dma:
                        nc.sync.dma_start(vall[p][:], vd[p])
                if do_cmp:
                    for c in range(nch):
                        cs = c * D
                        for p in gp:
                            mvj = vall[p][:, cs:cs + D]
                            if c > 0:
                                # carry from previous chunk: psum row 0
                                nc.scalar.copy(vall[p][0:1, cs:cs + D],
                                               ps[p][0:1, 0:128])
                            # scale rows by w (row 0 i.e. carry by gamma_c)
                            scale_eng.tensor_scalar_mul(mvj, mvj,
                                                        W[p][:, c:c + 1])
                            nc.tensor.matmul(ps[p][0:128, 0:128], u2[:], mvj,
                                             start=True, stop=True)
                            # alternate the PSUM-evacuating y-scale between
                            # ScalarE and VectorE to balance engine load
                            if (c + p) % 2 == 0:
                                nc.scalar.activation(yall[p][:, cs:cs + D],
                                                     ps[p][0:128, 0:128],
                                                     Act.Copy,
                                                     scale=R[p][:, c:c + 1])
                            else:
                                nc.vector.tensor_scalar_mul(
                                    yall[p][:, cs:cs + D],
                                    ps[p][0:128, 0:128], R[p][:, c:c + 1])
                if do_dma:
                    for p in gp:
                        src = yall[p][:] if do_cmp else vall[p][:]
                        nc.sync.dma_start(yd[p], src)

    nc.compile()
    return nc


def pack_inputs(x, n=N):
    """[pairs_total, n, D] -> [pairs_total, 128, nch*D] permuted+padded."""
    nch = -(-n // CH)
    m = x.shape[0]
    xp = np.zeros((m, nch * CH, D), np.float32)
    xp[:, :n] = x
    xp = xp.reshape(m, nch, CH, D).transpose(0, 2, 1, 3)  # [m, 127, nch, D]
    out = np.zeros((m, 128, nch, D), np.float32)
    out[:, 1:] = xp
    return np.ascontiguousarray(out.reshape(m, 128, nch * D))


def unpack_output(yp, n=N):
    """[pairs_total, 128, nch*D] -> [pairs_total, n, D]."""
    nch = -(-n // CH)
    m = yp.shape[0]
    yv = yp.reshape(m, 128, nch, D)[:, 1:]          # [m, 127, nch, D]
    yv = yv.transpose(0, 2, 1, 3).reshape(m, nch * CH, D)
    return np.ascontiguousarray(yv[:, :n])


_cached = {}


def _get_nc():
    if "nc" not in _cached:
        _cached["nc"] = build_nc()
    return _cached["nc"]


def run_on_hw(q, k, v, trace=False):
    """q,k,v: np [B,H,N,D] f32 -> (y [B,H,N,D], exec_time_ns or None)."""
    from concourse.bass_utils import run_bass_kernel_spmd

    nc = _get_nc()
    qp = pack_inputs(np.asarray(q, np.float32).reshape(B * H, N, D))
    kp = pack_inputs(np.asarray(k, np.float32).reshape(B * H, N, D))
    vp = pack_inputs(np.asarray(v, np.float32).reshape(B * H, N, D))
    in_maps = [
        {
            "q": qp[c * PAIRS:(c + 1) * PAIRS],
            "k": kp[c * PAIRS:(c + 1) * PAIRS],
            "v": vp[c * PAIRS:(c + 1) * PAIRS],
        }
        for c in range(NCORES)
    ]
    try:
        res = run_bass_kernel_spmd(nc, in_maps, list(range(NCORES)), trace=trace)
    except Exception:
        if not trace:
            raise
        import traceback
        traceback.print_exc()
        print("trace=True path failed; retrying without trace", file=sys.stderr)
        res = run_bass_kernel_spmd(nc, in_maps, list(range(NCORES)), trace=False)
    yp = np.concatenate([np.asarray(res.results[c]["y"]) for c in range(NCORES)],
                        axis=0)
    return unpack_output(yp).reshape(B, H, N, D), res.exec_time_ns


def kernel(q, k, v):
    y, _ = run_on_hw(q, k, v, trace=False)
    return y



# revision 18
# speedup vs baseline: 2.4534x; 2.4534x over previous
"""Trainium2 Bass kernel: Aaren-style online-softmax linear-attention scan.

Math (per (b,h) pair, per timestep t):
    alpha_t = q_t . k_t                       (scalar)
    y_t = sum_{s<=t} exp(alpha_s - C_t) v_s / sum_{s<=t} exp(alpha_s - C_t)
for any stabilizer C_t >= running max (the ratio is invariant). We use the
running *chunk* max M_c, mathematically identical to the reference.

Layout: host pre-permutes each (b,h) pair's [N, D] into
    x_perm[p, c, d] = x[c*128 + p, d]       (fp16 on the wire)
i.e. [128 partitions = in-chunk time, 32 chunks x 128 features]. N = 4096 =
32*128 exactly: no padding, no spare carry row.

Per pair:
  alpha: prod = q*k (DVE fp16 2x), A[p,c] = reduce_d (DVE, fp32 out).
  stats: chunk maxes via PE transpose + DVE max-reduce, running max via a
    1-row max-scan, W = exp(A - M) (fp32), gamma_c = exp(M_{c-1} - M_c).
  numerator carries WITHOUT a serial chain: wv = v * W (Pool, per chunk),
    per-chunk sums S_c[d] via one-column matmuls (stationary = wv chunk),
    ONE 128-partition tensor_tensor_scan C_c = gamma_c*C_{c-1} + S_c, carry
    rows CmS_c = C_c - S_c transposed once to [32, 128] and scattered into a
    block-diagonal-expanded crowsX[j, (g, j', d)] = CmS_{4g+j}[d] * (j==j')
    by four tiny SBUF->SBUF DMAs.
  denominator: cumw = U2 @ W plus rank-1 ones (x) (D_c - Sw_c) where D is a
    1-row mult/add scan; R = 1/d once per pair.
  chunk groups (4 chunks per PSUM bank, all independent):
    psum[t,(c,d)] = sum_s U2[s,t] wv[s,(c,d)]   (ONE 512-wide matmul)
    psum += ones4^T @ crowsX[:, g]              (ONE 512-wide rank-1 batch)
    y_c = psum_c * R[:,c]                       (4 Act evacs, fp16 out)

The emission is software-pipelined (phase1a(p) | chunks(p-1) | phase1b(p))
so every engine queue stays supplied without cross-pair stalls.

Sharding: B*H = 64 pairs -> 8 pairs per NeuronCore, no cross-core traffic.
fp16 wire halves HBM traffic; all accumulation stays fp32 (PSUM / DVE).
"""

import sys

for _p in ("/root/.axon_site/_ro/trn_rl_repo", "/opt/trn_rl_repo"):
    if _p not in sys.path:
        sys.path.append(_p)

import numpy as np

B, H, N, D = 4, 16, 4096, 128
NCORES = 8
PAIRS = B * H // NCORES  # 8 (b,h) pairs per core

CH = 128           # timesteps per chunk
NCH = N // CH      # 32 chunks
FW = NCH * D       # free width of the packed per-pair tiles (4096)
GW = 4 * D         # chunk-group width: 4 chunks per PSUM bank
NG = NCH // 4      # 8 chunk groups
NEG = -3.0e38


def build_nc(pairs=PAIRS, n=N, mode="full"):
    import concourse.tile as tile
    from concourse import bacc, mybir
    from contextlib import ExitStack

    do_dma = mode in ("full", "dma")
    do_cmp = mode in ("full", "compute")

    f16 = mybir.dt.float16
    bf16 = mybir.dt.bfloat16
    f32 = mybir.dt.float32
    Alu = mybir.AluOpType
    Act = mybir.ActivationFunctionType
    X = mybir.AxisListType.X

    nch = n // CH
    fw = nch * D
    ng = nch // 4

    nc = bacc.Bacc("TRN2", target_bir_lowering=False, debug=False)

    qkvd = nc.dram_tensor("qkv", [pairs, 128, 3 * fw], f16,
                          kind="ExternalInput")
    yd = nc.dram_tensor("y", [pairs, 128, fw], f16, kind="ExternalOutput")

    with tile.TileContext(nc) as tc, ExitStack() as ctx:
        cpool = ctx.enter_context(tc.tile_pool(name="consts", bufs=1))
        qkpool = ctx.enter_context(tc.tile_pool(name="qkv", bufs=4))
        prpool = ctx.enter_context(tc.tile_pool(name="prod", bufs=2))
        wvpool = ctx.enter_context(tc.tile_pool(name="wv", bufs=3))
        ypool = ctx.enter_context(tc.tile_pool(name="yy", bufs=2))
        smpool = ctx.enter_context(tc.tile_pool(name="sm", bufs=3))
        scpool = ctx.enter_context(
            tc.tile_pool(name="scr", bufs=3, space="PSUM"))
        pspool = ctx.enter_context(
            tc.tile_pool(name="ps", bufs=5, space="PSUM"))

        # ---- constants -------------------------------------------------
        iota_f = cpool.tile([128, 128], f32, tag="iotaf")
        nc.gpsimd.iota(iota_f[:], [[1, 128]], channel_multiplier=0,
                       allow_small_or_imprecise_dtypes=True)
        iota_p = cpool.tile([128, 1], f32, tag="iotap")
        nc.gpsimd.iota(iota_p[:], [[0, 1]], channel_multiplier=1,
                       allow_small_or_imprecise_dtypes=True)
        # u2[s, t] = 1.0 if t >= s else 0.0 (full lower-triangular)
        u2 = cpool.tile([128, 128], bf16, tag="u2")
        nc.vector.tensor_scalar(u2[:], iota_f[:], iota_p[:], None, Alu.is_ge)
        u2_32 = cpool.tile([128, 128], f32, tag="u2f32")
        nc.vector.tensor_scalar(u2_32[:], iota_f[:], iota_p[:], None,
                                Alu.is_ge)
        ident = cpool.tile([128, 128], f32, tag="ident")
        nc.vector.tensor_scalar(ident[:], iota_f[:], iota_p[:], None,
                                Alu.is_equal)
        ones_row32 = cpool.tile([1, 128], f32, tag="onesrow32")
        nc.gpsimd.memset(ones_row32[:], 1.0)
        ones_col32 = cpool.tile([128, 1], f32, tag="onescol32")
        nc.gpsimd.memset(ones_col32[:], 1.0)
        ones_col = cpool.tile([128, 1], bf16, tag="onescol")
        nc.gpsimd.memset(ones_col[:], 1.0)
        # SEL[s, c*128 + t] = 1.0 if s == c else 0: selector stationary used
        # to broadcast carry row c of crows to every output partition.
        jrep = cpool.tile([32, nch * 128], f32, tag="jrep")
        nc.gpsimd.iota(jrep[:], [[1, nch], [0, 128]], channel_multiplier=0,
                       allow_small_or_imprecise_dtypes=True)
        iota_p32 = cpool.tile([32, 1], f32, tag="iotap32")
        nc.gpsimd.iota(iota_p32[:], [[0, 1]], channel_multiplier=1,
                       allow_small_or_imprecise_dtypes=True)
        sel = cpool.tile([32, nch * 128], bf16, tag="sel")
        nc.vector.tensor_scalar(sel[:], jrep[:], iota_p32[:], None,
                                Alu.is_equal)

        qt, kt, vt, yt, wvt = {}, {}, {}, {}, {}
        Wt, Rt, gmt, crt = {}, {}, {}, {}
        scrt = {}

        def load(p):
            qkv = qkpool.tile([128, 3 * fw], f16, tag="qkv", name=f"qkv{p}")
            qt[p] = qkv[:, 0:fw]
            kt[p] = qkv[:, fw:2 * fw]
            vt[p] = qkv[:, 2 * fw:3 * fw]
            if do_dma:
                nc.sync.dma_start(qkv[:, 0:2 * fw], qkvd[p][:, 0:2 * fw])
                nc.sync.dma_start(qkv[:, 2 * fw:3 * fw],
                                  qkvd[p][:, 2 * fw:3 * fw])

        if not do_cmp:
            for p in range(pairs):
                load(p)
                if do_dma:
                    nc.sync.dma_start(yd[p], vt[p])
            nc.compile()
            return nc

        def phase1a(p):
            """Input DMA + alpha, chunk/running maxes, A-M, exps."""
            load(p)
            prod = prpool.tile([128, fw], f16, tag="pr", name=f"pr{p}")
            nc.vector.tensor_mul(prod[:], qt[p], kt[p])
            A = smpool.tile([128, nch], f32, tag="A", name=f"A{p}")
            nc.vector.tensor_reduce(
                A[:], prod[:].rearrange("p (c d) -> p c d", d=D),
                axis=X, op=Alu.add)

            scr = scpool.tile([128, 512], f32, tag="scr", name=f"scr{p}")
            scrt[p] = scr
            nc.tensor.transpose(scr[0:nch, 0:128], A[:], ident[:])
            mu = smpool.tile([128, 1], f32, tag="mu")
            nc.vector.tensor_reduce(mu[0:nch, :], scr[0:nch, 0:128],
                                    axis=X, op=Alu.max)
            nc.tensor.transpose(scr[0:1, 128:128 + nch], mu[0:nch, :],
                                ident[0:nch, 0:nch])
            mrow = smpool.tile([1, nch], f32, tag="mrow")
            nc.vector.tensor_copy(mrow[0:1, :], scr[0:1, 128:128 + nch])
            Mrow = smpool.tile([1, nch], f32, tag="Mrow")
            nc.vector.tensor_tensor_scan(Mrow[0:1, :], mrow[0:1, :],
                                         mrow[0:1, :], initial=NEG,
                                         op0=Alu.max, op1=Alu.max)
            # gamma logits: g2 = M_{c-1} - M_c (g2_0 = 0)
            g1 = smpool.tile([1, nch], f32, tag="g1")
            nc.vector.tensor_copy(g1[0:1, 1:nch], Mrow[0:1, 0:nch - 1])
            nc.vector.tensor_copy(g1[0:1, 0:1], Mrow[0:1, 0:1])
            g2 = smpool.tile([1, nch], f32, tag="g2", name=f"g2_{p}")
            nc.vector.tensor_sub(g2[0:1, :], g1[0:1, :], Mrow[0:1, :])
            # A - M broadcast
            nc.tensor.matmul(scr[0:128, 160:160 + nch], ones_row32[0:1, :],
                             Mrow[0:1, :], start=True, stop=True)
            AmM = smpool.tile([128, nch], f32, tag="AmM", name=f"AmM{p}")
            nc.vector.tensor_sub(AmM[:], A[:], scr[0:128, 160:160 + nch])
            W = smpool.tile([128, nch], f32, tag="W", name=f"W{p}")
            Wt[p] = W
            nc.scalar.activation(W[:], AmM[:], Act.Exp)
            gm = smpool.tile([1, nch], f32, tag="gm", name=f"gm{p}")
            gmt[p] = gm
            nc.scalar.activation(gm[0:1, :], g2[0:1, :], Act.Exp)

        def phase1b(p):
            """v scaling, denominators, carry chain, crowsX."""
            scr = scrt[p]
            W = Wt[p]
            gm = gmt[p]

            # scale v rows: wv = v * W[:, c] (Pool), freeing the qkv tile
            wv = wvpool.tile([128, fw], bf16, tag="wv", name=f"wv{p}")
            wvt[p] = wv
            for c in range(nch):
                cs = c * D
                nc.gpsimd.tensor_scalar_mul(wv[:, cs:cs + D],
                                            vt[p][:, cs:cs + D],
                                            W[:, c:c + 1])

            # denominator
            nc.tensor.matmul(scr[0:1, 200:200 + nch], ones_col32[:], W[:],
                             start=True, stop=True)
            swrow = smpool.tile([1, nch], f32, tag="swrow")
            nc.vector.tensor_copy(swrow[0:1, :], scr[0:1, 200:200 + nch])
            Drow = smpool.tile([1, nch], f32, tag="Drow")
            nc.vector.tensor_tensor_scan(Drow[0:1, :], gm[0:1, :],
                                         swrow[0:1, :], initial=0.0,
                                         op0=Alu.mult, op1=Alu.add)
            Dsh = smpool.tile([1, nch], f32, tag="Dsh")
            nc.vector.memset(Dsh[0:1, 0:1], 0.0)
            nc.vector.tensor_copy(Dsh[0:1, 1:nch], Drow[0:1, 0:nch - 1])
            adj = smpool.tile([1, nch], f32, tag="adj")
            nc.vector.tensor_mul(adj[0:1, :], gm[0:1, :], Dsh[0:1, :])
            dps = scr[0:128, 224:224 + nch]
            nc.tensor.matmul(dps, u2_32[:], W[:], start=True, stop=False)
            nc.tensor.matmul(dps, ones_row32[0:1, :], adj[0:1, :],
                             start=False, stop=True)
            R = smpool.tile([128, nch], f32, tag="R", name=f"R{p}")
            Rt[p] = R
            nc.vector.reciprocal(R[:], dps)

            # gamma broadcast for the 128-lane scan
            nc.tensor.matmul(scr[0:128, 256:256 + nch], ones_row32[0:1, :],
                             gm[0:1, :], start=True, stop=True)
            gb = smpool.tile([128, nch], f32, tag="gb")
            nc.vector.tensor_copy(gb[:], scr[0:128, 256:256 + nch])

            # numerator carries: per-chunk sums -> scan -> carry rows
            ST = scr[0:128, 288:288 + nch]
            for c in range(nch):
                nc.tensor.matmul(ST[:, c:c + 1], wv[:, c * D:(c + 1) * D],
                                 ones_col[:], start=True, stop=True)
            C = smpool.tile([128, nch], f32, tag="C")
            nc.vector.tensor_tensor_scan(C[:], gb[:], ST, initial=0.0,
                                         op0=Alu.mult, op1=Alu.add)
            Csh = smpool.tile([128, nch], f32, tag="Csh")
            nc.vector.memset(Csh[:, 0:1], 0.0)
            nc.vector.tensor_copy(Csh[:, 1:nch], C[:, 0:nch - 1])
            CmS = smpool.tile([128, nch], f32, tag="CmS")
            nc.vector.tensor_mul(CmS[:], gb[:], Csh[:])
            nc.tensor.transpose(scr[0:nch, 320:320 + 128], CmS[:], ident[:])
            crows = smpool.tile([nch, 128], bf16, tag="cr", name=f"cr{p}")
            crt[p] = crows
            nc.vector.tensor_copy(crows[:], scr[0:nch, 320:320 + 128])


        def chunks(p):
            """Batched prefix matmuls + per-chunk rank-1 carries + evacs."""
            wv, crows, R = wvt[p][:], crt[p], Rt[p]
            yt[p] = ypool.tile([128, fw], f16, tag="ya", name=f"ya{p}")
            for g in range(ng):
                gs = g * GW
                ps = pspool.tile([128, 512], f32, tag="cps",
                                 name=f"cps{p}_{g}")
                nc.tensor.matmul(ps[:, :], u2[:], wv[:, gs:gs + GW],
                                 start=True, stop=False)
                for j in range(4):
                    c = 4 * g + j
                    nc.tensor.matmul(ps[:, j * D:(j + 1) * D],
                                     sel[:, c * D:(c + 1) * D], crows[:, :],
                                     start=False, stop=(j == 3))
                for j in range(4):
                    c = 4 * g + j
                    nc.scalar.activation(yt[p][:, c * D:(c + 1) * D],
                                         ps[:, j * D:(j + 1) * D], Act.Copy,
                                         scale=R[:, c:c + 1])
            if do_dma:
                nc.sync.dma_start(yd[p], yt[p][:])

        for p in range(pairs + 1):
            if p < pairs:
                phase1a(p)
            if p >= 1:
                chunks(p - 1)
            if p < pairs:
                phase1b(p)

    nc.compile()
    return nc


def pack_inputs(x, n=N):
    """[pairs_total, n, D] f32 -> [pairs_total, 128, nch*D] fp16 permuted."""
    nch = n // CH
    m = x.shape[0]
    xp = x.reshape(m, nch, CH, D).transpose(0, 2, 1, 3)  # [m, 128, nch, D]
    return np.ascontiguousarray(xp.reshape(m, 128, nch * D).astype(np.float16))


def unpack_output(yp, n=N):
    """[pairs_total, 128, nch*D] fp16 -> [pairs_total, n, D] f32."""
    nch = n // CH
    m = yp.shape[0]
    yv = yp.astype(np.float32).reshape(m, 128, nch, D)
    yv = yv.transpose(0, 2, 1, 3).reshape(m, nch * CH, D)
    return np.ascontiguousarray(yv)


_cached = {}


def _get_nc():
    if "nc" not in _cached:
        _cached["nc"] = build_nc()
    return _cached["nc"]


def run_on_hw(q, k, v, trace=False):
    """q,k,v: np [B,H,N,D] f32 -> (y [B,H,N,D], exec_time_ns or None)."""
    from concourse.bass_utils import run_bass_kernel_spmd

    nc = _get_nc()
    qp = pack_inputs(np.asarray(q, np.float32).reshape(B * H, N, D))
    kp = pack_inputs(np.asarray(k, np.float32).reshape(B * H, N, D))
    vp = pack_inputs(np.asarray(v, np.float32).reshape(B * H, N, D))
    qkvp = np.ascontiguousarray(np.concatenate([qp, kp, vp], axis=2))
    in_maps = [
        {"qkv": qkvp[c * PAIRS:(c + 1) * PAIRS]}
        for c in range(NCORES)
    ]
    try:
        res = run_bass_kernel_spmd(nc, in_maps, list(range(NCORES)), trace=trace)
    except Exception:
        if not trace:
            raise
        import traceback
        traceback.print_exc()
        print("trace=True path failed; retrying without trace", file=sys.stderr)
        res = run_bass_kernel_spmd(nc, in_maps, list(range(NCORES)), trace=False)
    yp = np.concatenate([np.asarray(res.results[c]["y"]) for c in range(NCORES)],
                        axis=0)
    return unpack_output(yp).reshape(B, H, N, D), res.exec_time_ns


def kernel(q, k, v):
    y, _ = run_on_hw(q, k, v, trace=False)
    return y


# revision 23
# speedup vs baseline: 2.8719x; 1.1706x over previous
"""Trainium2 Bass kernel: Aaren-style online-softmax linear-attention scan.

Math (per (b,h) pair, per timestep t):
    alpha_t = q_t . k_t                       (scalar)
    y_t = sum_{s<=t} exp(alpha_s - C_t) v_s / sum_{s<=t} exp(alpha_s - C_t)
for any stabilizer C_t >= running max (the ratio is invariant). We use the
running *chunk* max M_c, mathematically identical to the reference.

Layout: host pre-permutes each (b,h) pair's [N, D] into
    x_perm[p, c, d] = x[c*128 + p, d]       (fp16 on the wire)
i.e. [128 partitions = in-chunk time, 32 chunks x 128 features]. N = 4096 =
32*128 exactly: no padding, no spare carry row.

Per pair:
  alpha: prod = q*k (DVE fp16 2x), A[p,c] = reduce_d (DVE, fp32 out).
  stats: chunk maxes via PE transpose + DVE max-reduce, running max via a
    1-row max-scan, W = exp(A - M) (fp32), gamma_c = exp(M_{c-1} - M_c).
  numerator carries WITHOUT a serial chain: wv = v * W (Pool, per chunk),
    per-chunk sums S_c[d] via one-column matmuls (stationary = wv chunk),
    ONE 128-partition tensor_tensor_scan C_c = gamma_c*C_{c-1} + S_c, carry
    rows CmS_c = C_c - S_c transposed once to [32, 128] and scattered into a
    block-diagonal-expanded crowsX[j, (g, j', d)] = CmS_{4g+j}[d] * (j==j')
    by four tiny SBUF->SBUF DMAs.
  denominator: cumw = U2 @ W plus rank-1 ones (x) (D_c - Sw_c) where D is a
    1-row mult/add scan; R = 1/d once per pair.
  chunk groups (4 chunks per PSUM bank, all independent):
    psum[t,(c,d)] = sum_s U2[s,t] wv[s,(c,d)]   (ONE 512-wide matmul)
    psum += ones4^T @ crowsX[:, g]              (ONE 512-wide rank-1 batch)
    y_c = psum_c * R[:,c]                       (4 Act evacs, fp16 out)

The emission is software-pipelined (phase1a(p) | chunks(p-1) | phase1b(p))
so every engine queue stays supplied without cross-pair stalls.

Sharding: B*H = 64 pairs -> 8 pairs per NeuronCore, no cross-core traffic.
fp16 wire halves HBM traffic; all accumulation stays fp32 (PSUM / DVE).
"""

import sys

for _p in ("/root/.axon_site/_ro/trn_rl_repo", "/opt/trn_rl_repo"):
    if _p not in sys.path:
        sys.path.append(_p)

import numpy as np

B, H, N, D = 4, 16, 4096, 128
NCORES = 8
PAIRS = B * H // NCORES  # 8 (b,h) pairs per core

CH = 128           # timesteps per chunk
NCH = N // CH      # 32 chunks
FW = NCH * D       # free width of the packed per-pair tiles (4096)
GW = 4 * D         # chunk-group width: 4 chunks per PSUM bank
NG = NCH // 4      # 8 chunk groups
NEG = -3.0e38


def build_nc(pairs=PAIRS, n=N, mode="full"):
    import concourse.tile as tile
    from concourse import bacc, mybir
    from concourse.bass import broadcast_tensor_aps
    from contextlib import ExitStack

    do_dma = mode in ("full", "dma")
    do_cmp = mode in ("full", "compute")

    f16 = mybir.dt.float16
    bf16 = mybir.dt.bfloat16
    f32 = mybir.dt.float32
    Alu = mybir.AluOpType
    Act = mybir.ActivationFunctionType
    X = mybir.AxisListType.X

    nch = n // CH
    fw = nch * D
    ng = nch // 4

    nc = bacc.Bacc("TRN2", target_bir_lowering=False, debug=False)

    qkvd = nc.dram_tensor("qkv", [pairs, 128, 3 * fw], f16,
                          kind="ExternalInput")
    yd = nc.dram_tensor("y", [pairs, 128, fw], f16, kind="ExternalOutput")

    with tile.TileContext(nc) as tc, ExitStack() as ctx:
        cpool = ctx.enter_context(tc.tile_pool(name="consts", bufs=1))
        qkpool = ctx.enter_context(tc.tile_pool(name="qkv", bufs=4))
        prpool = ctx.enter_context(tc.tile_pool(name="prod", bufs=2))
        wvpool = ctx.enter_context(tc.tile_pool(name="wv", bufs=3))
        ypool = ctx.enter_context(tc.tile_pool(name="yy", bufs=2))
        smpool = ctx.enter_context(tc.tile_pool(name="sm", bufs=3))
        scpool = ctx.enter_context(
            tc.tile_pool(name="scr", bufs=3, space="PSUM"))
        pspool = ctx.enter_context(
            tc.tile_pool(name="ps", bufs=5, space="PSUM"))

        # ---- constants -------------------------------------------------
        iota_f = cpool.tile([128, 128], f32, tag="iotaf")
        nc.gpsimd.iota(iota_f[:], [[1, 128]], channel_multiplier=0,
                       allow_small_or_imprecise_dtypes=True)
        iota_p = cpool.tile([128, 1], f32, tag="iotap")
        nc.gpsimd.iota(iota_p[:], [[0, 1]], channel_multiplier=1,
                       allow_small_or_imprecise_dtypes=True)
        # u2[s, t] = 1.0 if t >= s else 0.0 (full lower-triangular)
        u2 = cpool.tile([128, 128], bf16, tag="u2")
        nc.vector.tensor_scalar(u2[:], iota_f[:], iota_p[:], None, Alu.is_ge)
        u2_32 = cpool.tile([128, 128], f32, tag="u2f32")
        nc.vector.tensor_scalar(u2_32[:], iota_f[:], iota_p[:], None,
                                Alu.is_ge)
        ident = cpool.tile([128, 128], f32, tag="ident")
        nc.vector.tensor_scalar(ident[:], iota_f[:], iota_p[:], None,
                                Alu.is_equal)
        ones_row32 = cpool.tile([1, 128], f32, tag="onesrow32")
        nc.gpsimd.memset(ones_row32[:], 1.0)
        ones_col32 = cpool.tile([128, 1], f32, tag="onescol32")
        nc.gpsimd.memset(ones_col32[:], 1.0)
        ones_col = cpool.tile([128, 1], bf16, tag="onescol")
        nc.gpsimd.memset(ones_col[:], 1.0)
        # SEL[s, c*128 + t] = 1.0 if s == c else 0: selector stationary used
        # to broadcast carry row c of crows to every output partition.
        jrep = cpool.tile([32, nch * 128], f32, tag="jrep")
        nc.gpsimd.iota(jrep[:], [[1, nch], [0, 128]], channel_multiplier=0,
                       allow_small_or_imprecise_dtypes=True)
        iota_p32 = cpool.tile([32, 1], f32, tag="iotap32")
        nc.gpsimd.iota(iota_p32[:], [[0, 1]], channel_multiplier=1,
                       allow_small_or_imprecise_dtypes=True)
        sel = cpool.tile([32, nch * 128], bf16, tag="sel")
        nc.vector.tensor_scalar(sel[:], jrep[:], iota_p32[:], None,
                                Alu.is_equal)

        qt, kt, vt, yt, wvt = {}, {}, {}, {}, {}
        Wt, Rt, gmt, crt = {}, {}, {}, {}
        scrt = {}

        def load(p):
            qkv = qkpool.tile([128, 3 * fw], f16, tag="qkv", name=f"qkv{p}")
            qt[p] = qkv[:, 0:fw]
            kt[p] = qkv[:, fw:2 * fw]
            vt[p] = qkv[:, 2 * fw:3 * fw]
            if do_dma:
                nc.sync.dma_start(qkv[:, 0:2 * fw], qkvd[p][:, 0:2 * fw])
                nc.sync.dma_start(qkv[:, 2 * fw:3 * fw],
                                  qkvd[p][:, 2 * fw:3 * fw])

        if not do_cmp:
            for p in range(pairs):
                load(p)
                if do_dma:
                    nc.sync.dma_start(yd[p], vt[p])
            nc.compile()
            return nc

        def phase1a(p):
            """Input DMA + alpha, chunk/running maxes, A-M, exps."""
            load(p)
            prod = prpool.tile([128, fw], f16, tag="pr", name=f"pr{p}")
            nc.vector.tensor_mul(prod[:], qt[p], kt[p])
            A = smpool.tile([128, nch], f32, tag="A", name=f"A{p}")
            nc.vector.tensor_reduce(
                A[:], prod[:].rearrange("p (c d) -> p c d", d=D),
                axis=X, op=Alu.add)

            scr = scpool.tile([128, 512], f32, tag="scr", name=f"scr{p}")
            scrt[p] = scr
            nc.tensor.transpose(scr[0:nch, 0:128], A[:], ident[:])
            mu = smpool.tile([128, 1], f32, tag="mu")
            nc.vector.tensor_reduce(mu[0:nch, :], scr[0:nch, 0:128],
                                    axis=X, op=Alu.max)
            nc.tensor.transpose(scr[0:1, 128:128 + nch], mu[0:nch, :],
                                ident[0:nch, 0:nch])
            mrow = smpool.tile([1, nch], f32, tag="mrow")
            nc.vector.tensor_copy(mrow[0:1, :], scr[0:1, 128:128 + nch])
            Mrow = smpool.tile([1, nch], f32, tag="Mrow")
            nc.vector.tensor_tensor_scan(Mrow[0:1, :], mrow[0:1, :],
                                         mrow[0:1, :], initial=NEG,
                                         op0=Alu.max, op1=Alu.max)
            # gamma logits: g2 = M_{c-1} - M_c (g2_0 = 0)
            g1 = smpool.tile([1, nch], f32, tag="g1")
            nc.vector.tensor_copy(g1[0:1, 1:nch], Mrow[0:1, 0:nch - 1])
            nc.vector.tensor_copy(g1[0:1, 0:1], Mrow[0:1, 0:1])
            g2 = smpool.tile([1, nch], f32, tag="g2", name=f"g2_{p}")
            nc.vector.tensor_sub(g2[0:1, :], g1[0:1, :], Mrow[0:1, :])
            # A - M broadcast
            nc.tensor.matmul(scr[0:128, 160:160 + nch], ones_row32[0:1, :],
                             Mrow[0:1, :], start=True, stop=True)
            AmM = smpool.tile([128, nch], f32, tag="AmM", name=f"AmM{p}")
            nc.vector.tensor_sub(AmM[:], A[:], scr[0:128, 160:160 + nch])
            W = smpool.tile([128, nch], f32, tag="W", name=f"W{p}")
            Wt[p] = W
            nc.scalar.activation(W[:], AmM[:], Act.Exp)
            gm = smpool.tile([1, nch], f32, tag="gm", name=f"gm{p}")
            gmt[p] = gm
            nc.scalar.activation(gm[0:1, :], g2[0:1, :], Act.Exp)

        def phase1b(p):
            """v scaling, denominators, carry chain, crowsX."""
            scr = scrt[p]
            W = Wt[p]
            gm = gmt[p]

            # scale v rows: wv = v * W[:, c] (Pool), freeing the qkv tile
            wv = wvpool.tile([128, fw], bf16, tag="wv", name=f"wv{p}")
            wvt[p] = wv
            for c in range(nch):
                cs = c * D
                nc.gpsimd.tensor_scalar_mul(wv[:, cs:cs + D],
                                            vt[p][:, cs:cs + D],
                                            W[:, c:c + 1])

            # denominator
            nc.tensor.matmul(scr[0:1, 200:200 + nch], ones_col32[:], W[:],
                             start=True, stop=True)
            swrow = smpool.tile([1, nch], f32, tag="swrow")
            nc.vector.tensor_copy(swrow[0:1, :], scr[0:1, 200:200 + nch])
            Drow = smpool.tile([1, nch], f32, tag="Drow")
            nc.vector.tensor_tensor_scan(Drow[0:1, :], gm[0:1, :],
                                         swrow[0:1, :], initial=0.0,
                                         op0=Alu.mult, op1=Alu.add)
            Dsh = smpool.tile([1, nch], f32, tag="Dsh")
            nc.vector.memset(Dsh[0:1, 0:1], 0.0)
            nc.vector.tensor_copy(Dsh[0:1, 1:nch], Drow[0:1, 0:nch - 1])
            adj = smpool.tile([1, nch], f32, tag="adj")
            nc.vector.tensor_mul(adj[0:1, :], gm[0:1, :], Dsh[0:1, :])
            dps = scr[0:128, 224:224 + nch]
            nc.tensor.matmul(dps, u2_32[:], W[:], start=True, stop=False)
            nc.tensor.matmul(dps, ones_row32[0:1, :], adj[0:1, :],
                             start=False, stop=True)
            R = smpool.tile([128, nch], f32, tag="R", name=f"R{p}")
            Rt[p] = R
            nc.vector.reciprocal(R[:], dps)

            # gamma broadcast for the 128-lane scan
            nc.tensor.matmul(scr[0:128, 256:256 + nch], ones_row32[0:1, :],
                             gm[0:1, :], start=True, stop=True)
            gb = smpool.tile([128, nch], f32, tag="gb")
            nc.vector.tensor_copy(gb[:], scr[0:128, 256:256 + nch])

            # numerator carries: per-chunk sums -> scan -> carry rows
            ST = scr[0:128, 288:288 + nch]
            for c in range(nch):
                nc.tensor.matmul(ST[:, c:c + 1], wv[:, c * D:(c + 1) * D],
                                 ones_col[:], start=True, stop=True)
            C = smpool.tile([128, nch], f32, tag="C")
            nc.vector.tensor_tensor_scan(C[:], gb[:], ST, initial=0.0,
                                         op0=Alu.mult, op1=Alu.add)
            Csh = smpool.tile([128, nch], f32, tag="Csh")
            nc.vector.memset(Csh[:, 0:1], 0.0)
            nc.vector.tensor_copy(Csh[:, 1:nch], C[:, 0:nch - 1])
            CmS = smpool.tile([128, nch], f32, tag="CmS")
            nc.vector.tensor_mul(CmS[:], gb[:], Csh[:])
            nc.tensor.transpose(scr[0:nch, 320:320 + 128], CmS[:], ident[:])
            crows = smpool.tile([nch, 128], bf16, tag="cr", name=f"cr{p}")
            crt[p] = crows
            nc.vector.tensor_copy(crows[:], scr[0:nch, 320:320 + 128])


        def chunks(p):
            """Batched prefix matmuls + per-chunk rank-1 carries + evacs."""
            wv, crows, R = wvt[p][:], crt[p], Rt[p]
            yt[p] = ypool.tile([128, fw], f16, tag="ya", name=f"ya{p}")
            for g in range(ng):
                gs = g * GW
                ps = pspool.tile([128, 512], f32, tag="cps",
                                 name=f"cps{p}_{g}")
                nc.tensor.matmul(ps[:, :], u2[:], wv[:, gs:gs + GW],
                                 start=True, stop=False)
                for j in range(4):
                    c = 4 * g + j
                    nc.tensor.matmul(ps[:, j * D:(j + 1) * D],
                                     sel[:, c * D:(c + 1) * D], crows[:, :],
                                     start=False, stop=(j == 3))
                if g % 2 == 1:
                    # batched DVE evac: y_g = ps * R (R broadcast along d)
                    y3 = yt[p][:, gs:gs + GW].rearrange(
                        "p (b d) -> p b d", d=D)
                    ps3 = ps[:, :].rearrange("p (b d) -> p b d", d=D)
                    r3 = R[:, 4 * g:4 * g + 4].rearrange(
                        "p (b o) -> p b o", o=1)
                    r3b, ps3b = broadcast_tensor_aps(r3, ps3)
                    nc.vector.tensor_tensor(y3, ps3b, r3b, op=Alu.mult)
                else:
                    for j in range(4):
                        c = 4 * g + j
                        nc.scalar.activation(yt[p][:, c * D:(c + 1) * D],
                                             ps[:, j * D:(j + 1) * D],
                                             Act.Copy, scale=R[:, c:c + 1])
            if do_dma:
                nc.scalar.dma_start(yd[p], yt[p][:])

        for p in range(pairs + 1):
            if p < pairs:
                phase1a(p)
            if p >= 1:
                chunks(p - 1)
            if p < pairs:
                phase1b(p)

    nc.compile()
    return nc


def pack_inputs(x, n=N):
    """[pairs_total, n, D] f32 -> [pairs_total, 128, nch*D] fp16 permuted."""
    nch = n // CH
    m = x.shape[0]
    xp = x.reshape(m, nch, CH, D).transpose(0, 2, 1, 3)  # [m, 128, nch, D]
    return np.ascontiguousarray(xp.reshape(m, 128, nch * D).astype(np.float16))


def unpack_output(yp, n=N):
    """[pairs_total, 128, nch*D] fp16 -> [pairs_total, n, D] f32."""
    nch = n // CH
    m = yp.shape[0]
    yv = yp.astype(np.float32).reshape(m, 128, nch, D)
    yv = yv.transpose(0, 2, 1, 3).reshape(m, nch * CH, D)
    return np.ascontiguousarray(yv)


_cached = {}


def _get_nc():
    if "nc" not in _cached:
        _cached["nc"] = build_nc()
    return _cached["nc"]


def run_on_hw(q, k, v, trace=False):
    """q,k,v: np [B,H,N,D] f32 -> (y [B,H,N,D], exec_time_ns or None)."""
    from concourse.bass_utils import run_bass_kernel_spmd

    nc = _get_nc()
    qp = pack_inputs(np.asarray(q, np.float32).reshape(B * H, N, D))
    kp = pack_inputs(np.asarray(k, np.float32).reshape(B * H, N, D))
    vp = pack_inputs(np.asarray(v, np.float32).reshape(B * H, N, D))
    qkvp = np.ascontiguousarray(np.concatenate([qp, kp, vp], axis=2))
    in_maps = [
        {"qkv": qkvp[c * PAIRS:(c + 1) * PAIRS]}
        for c in range(NCORES)
    ]
    try:
        res = run_bass_kernel_spmd(nc, in_maps, list(range(NCORES)), trace=trace)
    except Exception:
        if not trace:
            raise
        import traceback
        traceback.print_exc()
        print("trace=True path failed; retrying without trace", file=sys.stderr)
        res = run_bass_kernel_spmd(nc, in_maps, list(range(NCORES)), trace=False)
    yp = np.concatenate([np.asarray(res.results[c]["y"]) for c in range(NCORES)],
                        axis=0)
    return unpack_output(yp).reshape(B, H, N, D), res.exec_time_ns


def kernel(q, k, v):
    y, _ = run_on_hw(q, k, v, trace=False)
    return y


# revision 44
# speedup vs baseline: 2.9968x; 1.0435x over previous
"""Trainium2 Bass kernel: Aaren-style online-softmax linear-attention scan.

Math (per (b,h) pair, per timestep t):
    alpha_t = q_t . k_t                       (scalar)
    y_t = sum_{s<=t} exp(alpha_s - C_t) v_s / sum_{s<=t} exp(alpha_s - C_t)
for any stabilizer C_t >= running max (the ratio is invariant). We use the
running *chunk* max M_c, mathematically identical to the reference.

Layout: host pre-permutes each (b,h) pair's [N, D] into
    x_perm[p, c, d] = x[c*128 + p, d]       (fp16 on the wire)
i.e. [128 partitions = in-chunk time, 32 chunks x 128 features]. N = 4096 =
32*128 exactly: no padding, no spare carry row.

Per pair:
  alpha: prod = q*k (DVE fp16 2x), A[p,c] = reduce_d (DVE, fp32 out).
  stats: chunk maxes via PE transpose + DVE max-reduce, running max via a
    1-row max-scan, W = exp(A - M) (fp32), gamma_c = exp(M_{c-1} - M_c).
  numerator carries WITHOUT a serial chain: wv = v * W (Pool, per chunk),
    per-chunk sums S_c[d] via one-column matmuls (stationary = wv chunk),
    ONE 128-partition tensor_tensor_scan C_c = gamma_c*C_{c-1} + S_c, carry
    rows CmS_c = C_c - S_c transposed once to [32, 128] and scattered into a
    block-diagonal-expanded crowsX[j, (g, j', d)] = CmS_{4g+j}[d] * (j==j')
    by four tiny SBUF->SBUF DMAs.
  denominator: cumw = U2 @ W plus rank-1 ones (x) (D_c - Sw_c) where D is a
    1-row mult/add scan; R = 1/d once per pair.
  chunk groups (4 chunks per PSUM bank, all independent):
    psum[t,(c,d)] = sum_s U2[s,t] wv[s,(c,d)]   (ONE 512-wide matmul)
    psum += ones4^T @ crowsX[:, g]              (ONE 512-wide rank-1 batch)
    y_c = psum_c * R[:,c]                       (4 Act evacs, fp16 out)

The emission is software-pipelined (phase1a(p) | chunks(p-1) | phase1b(p))
so every engine queue stays supplied without cross-pair stalls.

Sharding: B*H = 64 pairs -> 8 pairs per NeuronCore, no cross-core traffic.
fp16 wire halves HBM traffic; all accumulation stays fp32 (PSUM / DVE).
"""

import sys

for _p in ("/root/.axon_site/_ro/trn_rl_repo", "/opt/trn_rl_repo"):
    if _p not in sys.path:
        sys.path.append(_p)

import numpy as np

B, H, N, D = 4, 16, 4096, 128
NCORES = 8
PAIRS = B * H // NCORES  # 8 (b,h) pairs per core

CH = 128           # timesteps per chunk
NCH = N // CH      # 32 chunks
FW = NCH * D       # free width of the packed per-pair tiles (4096)
GW = 4 * D         # chunk-group width: 4 chunks per PSUM bank
NG = NCH // 4      # 8 chunk groups
NEG = -3.0e38


def build_nc(pairs=PAIRS, n=N, mode="full"):
    import concourse.tile as tile
    from concourse import bacc, mybir
    from concourse.bass import broadcast_tensor_aps
    from contextlib import ExitStack

    do_dma = mode in ("full", "dma")
    do_cmp = mode in ("full", "compute")

    f16 = mybir.dt.float16
    bf16 = mybir.dt.bfloat16
    f32 = mybir.dt.float32
    Alu = mybir.AluOpType
    Act = mybir.ActivationFunctionType
    X = mybir.AxisListType.X

    nch = n // CH
    fw = nch * D
    ng = nch // 4

    nc = bacc.Bacc("TRN2", target_bir_lowering=False, debug=False)

    qkvd = nc.dram_tensor("qkv", [pairs, 128, 3 * fw], f16,
                          kind="ExternalInput")
    yd = nc.dram_tensor("y", [pairs, 128, fw], f16, kind="ExternalOutput")

    with tile.TileContext(nc) as tc, ExitStack() as ctx:
        cpool = ctx.enter_context(tc.tile_pool(name="consts", bufs=1))
        qkpool = ctx.enter_context(tc.tile_pool(name="qkv", bufs=5))
        prpool = ctx.enter_context(tc.tile_pool(name="prod", bufs=2))
        wvpool = ctx.enter_context(tc.tile_pool(name="wv", bufs=4))
        ypool = ctx.enter_context(tc.tile_pool(name="yy", bufs=2))
        smpool = ctx.enter_context(tc.tile_pool(name="sm", bufs=3))
        scpool = ctx.enter_context(
            tc.tile_pool(name="scr", bufs=3, space="PSUM"))
        pspool = ctx.enter_context(
            tc.tile_pool(name="ps", bufs=5, space="PSUM"))

        # ---- constants -------------------------------------------------
        iota_f = cpool.tile([128, 128], f32, tag="iotaf")
        nc.gpsimd.iota(iota_f[:], [[1, 128]], channel_multiplier=0,
                       allow_small_or_imprecise_dtypes=True)
        iota_p = cpool.tile([128, 1], f32, tag="iotap")
        nc.gpsimd.iota(iota_p[:], [[0, 1]], channel_multiplier=1,
                       allow_small_or_imprecise_dtypes=True)
        # u2[s, t] = 1.0 if t >= s else 0.0 (full lower-triangular)
        u2 = cpool.tile([128, 128], bf16, tag="u2")
        nc.vector.tensor_scalar(u2[:], iota_f[:], iota_p[:], None, Alu.is_ge)
        u2_32 = cpool.tile([128, 128], f32, tag="u2f32")
        nc.vector.tensor_scalar(u2_32[:], iota_f[:], iota_p[:], None,
                                Alu.is_ge)
        ident = cpool.tile([128, 128], f32, tag="ident")
        nc.vector.tensor_scalar(ident[:], iota_f[:], iota_p[:], None,
                                Alu.is_equal)
        ones_row32 = cpool.tile([1, 128], f32, tag="onesrow32")
        nc.gpsimd.memset(ones_row32[:], 1.0)
        ones_col32 = cpool.tile([128, 1], f32, tag="onescol32")
        nc.gpsimd.memset(ones_col32[:], 1.0)
        ones_col = cpool.tile([128, 1], bf16, tag="onescol")
        nc.gpsimd.memset(ones_col[:], 1.0)
        # SEL[s, c*128 + t] = 1.0 if s == c else 0: selector stationary used
        # to broadcast carry row c of crows to every output partition.
        # jrep is startup-only scratch; it borrows a wv pool slot.
        jrep = wvpool.tile([32, nch * 128], bf16, tag="wv", name="jrep")
        nc.gpsimd.iota(jrep[:], [[1, nch], [0, 128]], channel_multiplier=0,
                       allow_small_or_imprecise_dtypes=True)
        iota_p32 = cpool.tile([32, 1], f32, tag="iotap32")
        nc.gpsimd.iota(iota_p32[:], [[0, 1]], channel_multiplier=1,
                       allow_small_or_imprecise_dtypes=True)
        sel = cpool.tile([32, nch * 128], bf16, tag="sel")
        nc.gpsimd.tensor_scalar(sel[:], jrep[:], iota_p32[:], None,
                                Alu.is_equal)

        qt, kt, vt, yt, wvt = {}, {}, {}, {}, {}
        Wt, Rt, gmt, crt = {}, {}, {}, {}
        scrt = {}

        def load(p):
            qkv = qkpool.tile([128, 3 * fw], f16, tag="qkv", name=f"qkv{p}")
            qt[p] = qkv[:, 0:fw]
            kt[p] = qkv[:, fw:2 * fw]
            vt[p] = qkv[:, 2 * fw:3 * fw]
            if do_dma:
                nc.sync.dma_start(qkv[:, 0:2 * fw], qkvd[p][:, 0:2 * fw])
                nc.sync.dma_start(qkv[:, 2 * fw:3 * fw],
                                  qkvd[p][:, 2 * fw:3 * fw])

        if not do_cmp:
            for p in range(pairs):
                load(p)
                if do_dma:
                    nc.sync.dma_start(yd[p], vt[p])
            nc.compile()
            return nc

        def phase1a(p):
            """Input DMA + alpha, chunk/running maxes, A-M, exps."""
            load(p)
            prod = prpool.tile([128, fw], f16, tag="pr", name=f"pr{p}")
            nc.vector.tensor_mul(prod[:], qt[p], kt[p])
            A = smpool.tile([128, nch], f32, tag="A", name=f"A{p}")
            nc.vector.tensor_reduce(
                A[:], prod[:].rearrange("p (c d) -> p c d", d=D),
                axis=X, op=Alu.add)

            scr = scpool.tile([128, 512], f32, tag="scr", name=f"scr{p}")
            scrt[p] = scr
            nc.tensor.transpose(scr[0:nch, 0:128], A[:], ident[:])
            mu = smpool.tile([128, 1], f32, tag="mu")
            nc.vector.tensor_reduce(mu[0:nch, :], scr[0:nch, 0:128],
                                    axis=X, op=Alu.max)
            nc.tensor.transpose(scr[0:1, 128:128 + nch], mu[0:nch, :],
                                ident[0:nch, 0:nch])
            mrow = smpool.tile([1, nch], f32, tag="mrow")
            nc.vector.tensor_copy(mrow[0:1, :], scr[0:1, 128:128 + nch])
            Mrow = smpool.tile([1, nch], f32, tag="Mrow")
            nc.vector.tensor_tensor_scan(Mrow[0:1, :], mrow[0:1, :],
                                         mrow[0:1, :], initial=NEG,
                                         op0=Alu.max, op1=Alu.max)
            # gamma logits: g2 = M_{c-1} - M_c (g2_0 = 0)
            g1 = smpool.tile([1, nch], f32, tag="g1")
            nc.vector.tensor_copy(g1[0:1, 1:nch], Mrow[0:1, 0:nch - 1])
            nc.vector.tensor_copy(g1[0:1, 0:1], Mrow[0:1, 0:1])
            g2 = smpool.tile([1, nch], f32, tag="g2", name=f"g2_{p}")
            nc.vector.tensor_sub(g2[0:1, :], g1[0:1, :], Mrow[0:1, :])
            # A - M broadcast
            nc.tensor.matmul(scr[0:128, 160:160 + nch], ones_row32[0:1, :],
                             Mrow[0:1, :], start=True, stop=True)
            AmM = smpool.tile([128, nch], f32, tag="AmM", name=f"AmM{p}")
            nc.vector.tensor_sub(AmM[:], A[:], scr[0:128, 160:160 + nch])
            W = smpool.tile([128, nch], f32, tag="W", name=f"W{p}")
            Wt[p] = W
            nc.scalar.activation(W[:], AmM[:], Act.Exp)
            gm = smpool.tile([1, nch], f32, tag="gm", name=f"gm{p}")
            gmt[p] = gm
            nc.scalar.activation(gm[0:1, :], g2[0:1, :], Act.Exp)

        def phase1b(p):
            """v scaling, denominators, carry chain, crowsX."""
            scr = scrt[p]
            W = Wt[p]
            gm = gmt[p]

            # scale v rows: wv = v * W[:, c] (Pool), freeing the qkv tile
            wv = wvpool.tile([128, fw], bf16, tag="wv", name=f"wv{p}")
            wvt[p] = wv
            edge = p == pairs - 1
            for c in range(nch):
                cs = c * D
                eng = nc.vector if (edge and c % 2 == 1) else nc.gpsimd
                eng.tensor_scalar_mul(wv[:, cs:cs + D],
                                      vt[p][:, cs:cs + D],
                                      W[:, c:c + 1])

            # denominator
            nc.tensor.matmul(scr[0:1, 200:200 + nch], ones_col32[:], W[:],
                             start=True, stop=True)
            swrow = smpool.tile([1, nch], f32, tag="swrow")
            nc.vector.tensor_copy(swrow[0:1, :], scr[0:1, 200:200 + nch])
            Drow = smpool.tile([1, nch], f32, tag="Drow")
            nc.vector.tensor_tensor_scan(Drow[0:1, :], gm[0:1, :],
                                         swrow[0:1, :], initial=0.0,
                                         op0=Alu.mult, op1=Alu.add)
            Dsh = smpool.tile([1, nch], f32, tag="Dsh")
            nc.vector.memset(Dsh[0:1, 0:1], 0.0)
            nc.vector.tensor_copy(Dsh[0:1, 1:nch], Drow[0:1, 0:nch - 1])
            adj = smpool.tile([1, nch], f32, tag="adj")
            nc.vector.tensor_mul(adj[0:1, :], gm[0:1, :], Dsh[0:1, :])
            dps = scr[0:128, 224:224 + nch]
            nc.tensor.matmul(dps, u2_32[:], W[:], start=True, stop=False)
            nc.tensor.matmul(dps, ones_row32[0:1, :], adj[0:1, :],
                             start=False, stop=True)
            R = smpool.tile([128, nch], f32, tag="R", name=f"R{p}")
            Rt[p] = R
            nc.vector.reciprocal(R[:], dps)

            # gamma broadcast for the 128-lane scan
            nc.tensor.matmul(scr[0:128, 256:256 + nch], ones_row32[0:1, :],
                             gm[0:1, :], start=True, stop=True)
            gb = smpool.tile([128, nch], f32, tag="gb")
            nc.vector.tensor_copy(gb[:], scr[0:128, 256:256 + nch])

            # numerator carries: per-chunk sums -> scan -> carry rows
            ST = scr[0:128, 288:288 + nch]
            for c in range(nch):
                nc.tensor.matmul(ST[:, c:c + 1], wv[:, c * D:(c + 1) * D],
                                 ones_col[:], start=True, stop=True)
            C = smpool.tile([128, nch], f32, tag="C")
            nc.vector.tensor_tensor_scan(C[:], gb[:], ST, initial=0.0,
                                         op0=Alu.mult, op1=Alu.add)
            Csh = smpool.tile([128, nch], f32, tag="Csh")
            nc.vector.memset(Csh[:, 0:1], 0.0)
            nc.vector.tensor_copy(Csh[:, 1:nch], C[:, 0:nch - 1])
            CmS = smpool.tile([128, nch], f32, tag="CmS")
            nc.vector.tensor_mul(CmS[:], gb[:], Csh[:])
            nc.tensor.transpose(scr[0:nch, 320:320 + 128], CmS[:], ident[:])
            crows = smpool.tile([nch, 128], bf16, tag="cr", name=f"cr{p}")
            crt[p] = crows
            nc.vector.tensor_copy(crows[:], scr[0:nch, 320:320 + 128])


        def chunks(p):
            """Batched prefix matmuls + per-chunk rank-1 carries + evacs."""
            wv, crows, R = wvt[p][:], crt[p], Rt[p]
            yt[p] = ypool.tile([128, fw], f16, tag="ya", name=f"ya{p}")
            for g in range(ng):
                gs = g * GW
                ps = pspool.tile([128, 512], f32, tag="cps",
                                 name=f"cps{p}_{g}")
                nc.tensor.matmul(ps[:, :], u2[:], wv[:, gs:gs + GW],
                                 start=True, stop=False)
                for j in range(4):
                    c = 4 * g + j
                    nc.tensor.matmul(ps[:, j * D:(j + 1) * D],
                                     sel[:, c * D:(c + 1) * D], crows[:, :],
                                     start=False, stop=(j == 3))
                if g % 2 == 1:
                    # batched DVE evac: y_g = ps * R (R broadcast along d)
                    y3 = yt[p][:, gs:gs + GW].rearrange(
                        "p (b d) -> p b d", d=D)
                    ps3 = ps[:, :].rearrange("p (b d) -> p b d", d=D)
                    r3 = R[:, 4 * g:4 * g + 4].rearrange(
                        "p (b o) -> p b o", o=1)
                    r3b, ps3b = broadcast_tensor_aps(r3, ps3)
                    nc.vector.tensor_tensor(y3, ps3b, r3b, op=Alu.mult)
                else:
                    for j in range(4):
                        c = 4 * g + j
                        nc.scalar.activation(yt[p][:, c * D:(c + 1) * D],
                                             ps[:, j * D:(j + 1) * D],
                                             Act.Copy, scale=R[:, c:c + 1])
            if do_dma:
                if p == pairs - 1:
                    nc.scalar.dma_start(yd[p][:, 0:fw // 2],
                                        yt[p][:, 0:fw // 2])
                    nc.scalar.dma_start(yd[p][:, fw // 2:fw],
                                        yt[p][:, fw // 2:fw])
                else:
                    nc.scalar.dma_start(yd[p], yt[p][:])

        for p in range(pairs + 1):
            if p < pairs:
                phase1a(p)
            if p >= 1:
                chunks(p - 1)
            if p < pairs:
                phase1b(p)

    nc.compile()
    return nc


def pack_inputs(x, n=N):
    """[pairs_total, n, D] f32 -> [pairs_total, 128, nch*D] fp16 permuted."""
    nch = n // CH
    m = x.shape[0]
    xp = x.reshape(m, nch, CH, D).transpose(0, 2, 1, 3)  # [m, 128, nch, D]
    return np.ascontiguousarray(xp.reshape(m, 128, nch * D).astype(np.float16))


def unpack_output(yp, n=N):
    """[pairs_total, 128, nch*D] fp16 -> [pairs_total, n, D] f32."""
    nch = n // CH
    m = yp.shape[0]
    yv = yp.astype(np.float32).reshape(m, 128, nch, D)
    yv = yv.transpose(0, 2, 1, 3).reshape(m, nch * CH, D)
    return np.ascontiguousarray(yv)


_cached = {}


def _get_nc():
    if "nc" not in _cached:
        _cached["nc"] = build_nc()
    return _cached["nc"]


def run_on_hw(q, k, v, trace=False):
    """q,k,v: np [B,H,N,D] f32 -> (y [B,H,N,D], exec_time_ns or None)."""
    from concourse.bass_utils import run_bass_kernel_spmd

    nc = _get_nc()
    qp = pack_inputs(np.asarray(q, np.float32).reshape(B * H, N, D))
    kp = pack_inputs(np.asarray(k, np.float32).reshape(B * H, N, D))
    vp = pack_inputs(np.asarray(v, np.float32).reshape(B * H, N, D))
    qkvp = np.ascontiguousarray(np.concatenate([qp, kp, vp], axis=2))
    in_maps = [
        {"qkv": qkvp[c * PAIRS:(c + 1) * PAIRS]}
        for c in range(NCORES)
    ]
    try:
        res = run_bass_kernel_spmd(nc, in_maps, list(range(NCORES)), trace=trace)
    except Exception:
        if not trace:
            raise
        import traceback
        traceback.print_exc()
        print("trace=True path failed; retrying without trace", file=sys.stderr)
        res = run_bass_kernel_spmd(nc, in_maps, list(range(NCORES)), trace=False)
    yp = np.concatenate([np.asarray(res.results[c]["y"]) for c in range(NCORES)],
                        axis=0)
    return unpack_output(yp).reshape(B, H, N, D), res.exec_time_ns


def kernel(q, k, v):
    y, _ = run_on_hw(q, k, v, trace=False)
    return y


# revision 51
# speedup vs baseline: 3.0227x; 1.0086x over previous
"""Trainium2 Bass kernel: Aaren-style online-softmax linear-attention scan.

Math (per (b,h) pair, per timestep t):
    alpha_t = q_t . k_t                       (scalar)
    y_t = sum_{s<=t} exp(alpha_s - C_t) v_s / sum_{s<=t} exp(alpha_s - C_t)
for any stabilizer C_t >= running max (the ratio is invariant). We use the
running *chunk* max M_c, mathematically identical to the reference.

Layout: host pre-permutes each (b,h) pair's [N, D] into
    x_perm[p, c, d] = x[c*128 + p, d]       (fp16 on the wire)
i.e. [128 partitions = in-chunk time, 32 chunks x 128 features]. N = 4096 =
32*128 exactly: no padding, no spare carry row.

Per pair:
  alpha: prod = q*k (DVE fp16 2x), A[p,c] = reduce_d (DVE, fp32 out).
  stats: chunk maxes via PE transpose + DVE max-reduce, running max via a
    1-row max-scan, W = exp(A - M) (fp32), gamma_c = exp(M_{c-1} - M_c).
  numerator carries WITHOUT a serial chain: wv = v * W (Pool, per chunk),
    per-chunk sums S_c[d] via one-column matmuls (stationary = wv chunk),
    ONE 128-partition tensor_tensor_scan C_c = gamma_c*C_{c-1} + S_c, carry
    rows CmS_c = C_c - S_c transposed once to [32, 128] and scattered into a
    block-diagonal-expanded crowsX[j, (g, j', d)] = CmS_{4g+j}[d] * (j==j')
    by four tiny SBUF->SBUF DMAs.
  denominator: cumw = U2 @ W plus rank-1 ones (x) (D_c - Sw_c) where D is a
    1-row mult/add scan; R = 1/d once per pair.
  chunk groups (4 chunks per PSUM bank, all independent):
    psum[t,(c,d)] = sum_s U2[s,t] wv[s,(c,d)]   (ONE 512-wide matmul)
    psum += ones4^T @ crowsX[:, g]              (ONE 512-wide rank-1 batch)
    y_c = psum_c * R[:,c]                       (4 Act evacs, fp16 out)

The emission is software-pipelined (phase1a(p) | chunks(p-1) | phase1b(p))
so every engine queue stays supplied without cross-pair stalls.

Sharding: B*H = 64 pairs -> 8 pairs per NeuronCore, no cross-core traffic.
fp16 wire halves HBM traffic; all accumulation stays fp32 (PSUM / DVE).
"""

import sys

for _p in ("/root/.axon_site/_ro/trn_rl_repo", "/opt/trn_rl_repo"):
    if _p not in sys.path:
        sys.path.append(_p)

import numpy as np

B, H, N, D = 4, 16, 4096, 128
NCORES = 8
PAIRS = B * H // NCORES  # 8 (b,h) pairs per core

CH = 128           # timesteps per chunk
NCH = N // CH      # 32 chunks
FW = NCH * D       # free width of the packed per-pair tiles (4096)
GW = 4 * D         # chunk-group width: 4 chunks per PSUM bank
NG = NCH // 4      # 8 chunk groups
NEG = -3.0e38


def build_nc(pairs=PAIRS, n=N, mode="full"):
    import concourse.tile as tile
    from concourse import bacc, mybir
    from concourse.bass import broadcast_tensor_aps
    from contextlib import ExitStack

    do_dma = mode in ("full", "dma")
    do_cmp = mode in ("full", "compute")

    f16 = mybir.dt.float16
    bf16 = mybir.dt.bfloat16
    f32 = mybir.dt.float32
    Alu = mybir.AluOpType
    Act = mybir.ActivationFunctionType
    X = mybir.AxisListType.X

    nch = n // CH
    fw = nch * D
    ng = nch // 4

    nc = bacc.Bacc("TRN2", target_bir_lowering=False, debug=False)

    qkvd = nc.dram_tensor("qkv", [pairs, 128, 3 * fw], f16,
                          kind="ExternalInput")
    yd = nc.dram_tensor("y", [pairs, 128, fw], f16, kind="ExternalOutput")

    with tile.TileContext(nc) as tc, ExitStack() as ctx:
        cpool = ctx.enter_context(tc.tile_pool(name="consts", bufs=1))
        qkpool = ctx.enter_context(tc.tile_pool(name="qkv", bufs=5))
        prpool = ctx.enter_context(tc.tile_pool(name="prod", bufs=2))
        wvpool = ctx.enter_context(tc.tile_pool(name="wv", bufs=4))
        ypool = ctx.enter_context(tc.tile_pool(name="yy", bufs=2))
        smpool = ctx.enter_context(tc.tile_pool(name="sm", bufs=3))
        scpool = ctx.enter_context(
            tc.tile_pool(name="scr", bufs=3, space="PSUM"))
        pspool = ctx.enter_context(
            tc.tile_pool(name="ps", bufs=5, space="PSUM"))

        # ---- constants -------------------------------------------------
        iota_f = cpool.tile([128, 128], f32, tag="iotaf")
        nc.gpsimd.iota(iota_f[:], [[1, 128]], channel_multiplier=0,
                       allow_small_or_imprecise_dtypes=True)
        iota_p = cpool.tile([128, 1], f32, tag="iotap")
        nc.gpsimd.iota(iota_p[:], [[0, 1]], channel_multiplier=1,
                       allow_small_or_imprecise_dtypes=True)
        # u2[s, t] = 1.0 if t >= s else 0.0 (full lower-triangular)
        u2 = cpool.tile([128, 128], bf16, tag="u2")
        nc.vector.tensor_scalar(u2[:], iota_f[:], iota_p[:], None, Alu.is_ge)
        u2_32 = cpool.tile([128, 128], f32, tag="u2f32")
        nc.vector.tensor_scalar(u2_32[:], iota_f[:], iota_p[:], None,
                                Alu.is_ge)
        ident = cpool.tile([128, 128], f32, tag="ident")
        nc.vector.tensor_scalar(ident[:], iota_f[:], iota_p[:], None,
                                Alu.is_equal)
        ones_row32 = cpool.tile([1, 128], f32, tag="onesrow32")
        nc.gpsimd.memset(ones_row32[:], 1.0)
        ones_col32 = cpool.tile([128, 1], f32, tag="onescol32")
        nc.gpsimd.memset(ones_col32[:], 1.0)
        ones_col = cpool.tile([128, 1], bf16, tag="onescol")
        nc.gpsimd.memset(ones_col[:], 1.0)
        # SEL[s, c*128 + t] = 1.0 if s == c else 0: selector stationary used
        # to broadcast carry row c of crows to every output partition.
        # jrep is startup-only scratch; it borrows a wv pool slot.
        jrep = wvpool.tile([32, nch * 128], bf16, tag="wv", name="jrep")
        nc.gpsimd.iota(jrep[:], [[1, nch], [0, 128]], channel_multiplier=0,
                       allow_small_or_imprecise_dtypes=True)
        iota_p32 = cpool.tile([32, 1], f32, tag="iotap32")
        nc.gpsimd.iota(iota_p32[:], [[0, 1]], channel_multiplier=1,
                       allow_small_or_imprecise_dtypes=True)
        sel = cpool.tile([32, nch * 128], bf16, tag="sel")
        nc.gpsimd.tensor_scalar(sel[:], jrep[:], iota_p32[:], None,
                                Alu.is_equal)

        qt, kt, vt, yt, wvt = {}, {}, {}, {}, {}
        Wt, Rt, gmt, crt = {}, {}, {}, {}
        scrt = {}

        def load(p):
            qkv = qkpool.tile([128, 3 * fw], f16, tag="qkv", name=f"qkv{p}")
            qt[p] = qkv[:, 0:fw]
            kt[p] = qkv[:, fw:2 * fw]
            vt[p] = qkv[:, 2 * fw:3 * fw]
            if do_dma:
                nc.sync.dma_start(qkv[:, 0:2 * fw], qkvd[p][:, 0:2 * fw])
                nc.sync.dma_start(qkv[:, 2 * fw:3 * fw],
                                  qkvd[p][:, 2 * fw:3 * fw])

        if not do_cmp:
            for p in range(pairs):
                load(p)
                if do_dma:
                    nc.sync.dma_start(yd[p], vt[p])
            nc.compile()
            return nc

        def phase1a(p):
            """Input DMA + alpha, chunk/running maxes, A-M, exps."""
            load(p)
            prod = prpool.tile([128, fw], f16, tag="pr", name=f"pr{p}")
            nc.vector.tensor_mul(prod[:], qt[p], kt[p])
            A = smpool.tile([128, nch], f32, tag="A", name=f"A{p}")
            nc.vector.tensor_reduce(
                A[:], prod[:].rearrange("p (c d) -> p c d", d=D),
                axis=X, op=Alu.add)

            scr = scpool.tile([128, 512], f32, tag="scr", name=f"scr{p}")
            scrt[p] = scr
            nc.tensor.transpose(scr[0:nch, 0:128], A[:], ident[:])
            mu = smpool.tile([128, 1], f32, tag="mu")
            nc.vector.tensor_reduce(mu[0:nch, :], scr[0:nch, 0:128],
                                    axis=X, op=Alu.max)
            nc.tensor.transpose(scr[0:1, 128:128 + nch], mu[0:nch, :],
                                ident[0:nch, 0:nch])
            mrow = smpool.tile([1, nch], f32, tag="mrow")
            nc.vector.tensor_copy(mrow[0:1, :], scr[0:1, 128:128 + nch])
            Mrow = smpool.tile([1, nch], f32, tag="Mrow")
            nc.vector.tensor_tensor_scan(Mrow[0:1, :], mrow[0:1, :],
                                         mrow[0:1, :], initial=NEG,
                                         op0=Alu.max, op1=Alu.max)
            # gamma logits: g2 = M_{c-1} - M_c (g2_0 = 0)
            g1 = smpool.tile([1, nch], f32, tag="g1")
            nc.vector.tensor_copy(g1[0:1, 1:nch], Mrow[0:1, 0:nch - 1])
            nc.vector.tensor_copy(g1[0:1, 0:1], Mrow[0:1, 0:1])
            g2 = smpool.tile([1, nch], f32, tag="g2", name=f"g2_{p}")
            nc.vector.tensor_sub(g2[0:1, :], g1[0:1, :], Mrow[0:1, :])
            # A - M broadcast
            nc.tensor.matmul(scr[0:128, 160:160 + nch], ones_row32[0:1, :],
                             Mrow[0:1, :], start=True, stop=True)
            AmM = smpool.tile([128, nch], f32, tag="AmM", name=f"AmM{p}")
            nc.vector.tensor_sub(AmM[:], A[:], scr[0:128, 160:160 + nch])
            W = smpool.tile([128, nch], f32, tag="W", name=f"W{p}")
            Wt[p] = W
            nc.scalar.activation(W[:], AmM[:], Act.Exp)
            gm = smpool.tile([1, nch], f32, tag="gm", name=f"gm{p}")
            gmt[p] = gm
            nc.scalar.activation(gm[0:1, :], g2[0:1, :], Act.Exp)

        def phase1b(p):
            """v scaling, denominators, carry chain, crowsX."""
            scr = scrt[p]
            W = Wt[p]
            gm = gmt[p]

            # scale v rows: wv = v * W[:, c] (Pool), freeing the qkv tile
            wv = wvpool.tile([128, fw], bf16, tag="wv", name=f"wv{p}")
            wvt[p] = wv
            for c in range(nch):
                cs = c * D
                if p == pairs - 1 and c % 2 == 1:
                    eng = nc.vector
                elif p == pairs - 2 and c % 4 == 1:
                    eng = nc.vector
                else:
                    eng = nc.gpsimd
                eng.tensor_scalar_mul(wv[:, cs:cs + D],
                                      vt[p][:, cs:cs + D],
                                      W[:, c:c + 1])

            # denominator
            nc.tensor.matmul(scr[0:1, 200:200 + nch], ones_col32[:], W[:],
                             start=True, stop=True)
            swrow = smpool.tile([1, nch], f32, tag="swrow")
            nc.vector.tensor_copy(swrow[0:1, :], scr[0:1, 200:200 + nch])
            Drow = smpool.tile([1, nch], f32, tag="Drow")
            nc.vector.tensor_tensor_scan(Drow[0:1, :], gm[0:1, :],
                                         swrow[0:1, :], initial=0.0,
                                         op0=Alu.mult, op1=Alu.add)
            Dsh = smpool.tile([1, nch], f32, tag="Dsh")
            nc.vector.memset(Dsh[0:1, 0:1], 0.0)
            nc.vector.tensor_copy(Dsh[0:1, 1:nch], Drow[0:1, 0:nch - 1])
            adj = smpool.tile([1, nch], f32, tag="adj")
            nc.vector.tensor_mul(adj[0:1, :], gm[0:1, :], Dsh[0:1, :])
            dps = scr[0:128, 224:224 + nch]
            nc.tensor.matmul(dps, u2_32[:], W[:], start=True, stop=False)
            nc.tensor.matmul(dps, ones_row32[0:1, :], adj[0:1, :],
                             start=False, stop=True)
            R = smpool.tile([128, nch], f32, tag="R", name=f"R{p}")
            Rt[p] = R
            nc.vector.reciprocal(R[:], dps)

            # gamma broadcast for the 128-lane scan
            nc.tensor.matmul(scr[0:128, 256:256 + nch], ones_row32[0:1, :],
                             gm[0:1, :], start=True, stop=True)
            gb = smpool.tile([128, nch], f32, tag="gb")
            nc.vector.tensor_copy(gb[:], scr[0:128, 256:256 + nch])

            # numerator carries: per-chunk sums -> scan -> carry rows
            ST = scr[0:128, 288:288 + nch]
            for c in range(nch):
                nc.tensor.matmul(ST[:, c:c + 1], wv[:, c * D:(c + 1) * D],
                                 ones_col[:], start=True, stop=True)
            C = smpool.tile([128, nch], f32, tag="C")
            nc.vector.tensor_tensor_scan(C[:], gb[:], ST, initial=0.0,
                                         op0=Alu.mult, op1=Alu.add)
            Csh = smpool.tile([128, nch], f32, tag="Csh")
            nc.vector.memset(Csh[:, 0:1], 0.0)
            nc.vector.tensor_copy(Csh[:, 1:nch], C[:, 0:nch - 1])
            CmS = smpool.tile([128, nch], f32, tag="CmS")
            nc.vector.tensor_mul(CmS[:], gb[:], Csh[:])
            nc.tensor.transpose(scr[0:nch, 320:320 + 128], CmS[:], ident[:])
            crows = smpool.tile([nch, 128], bf16, tag="cr", name=f"cr{p}")
            crt[p] = crows
            nc.vector.tensor_copy(crows[:], scr[0:nch, 320:320 + 128])


        def chunks(p):
            """Batched prefix matmuls + per-chunk rank-1 carries + evacs."""
            wv, crows, R = wvt[p][:], crt[p], Rt[p]
            yt[p] = ypool.tile([128, fw], f16, tag="ya", name=f"ya{p}")
            for g in range(ng):
                gs = g * GW
                ps = pspool.tile([128, 512], f32, tag="cps",
                                 name=f"cps{p}_{g}")
                nc.tensor.matmul(ps[:, :], u2[:], wv[:, gs:gs + GW],
                                 start=True, stop=False)
                for j in range(4):
                    c = 4 * g + j
                    nc.tensor.matmul(ps[:, j * D:(j + 1) * D],
                                     sel[:, c * D:(c + 1) * D], crows[:, :],
                                     start=False, stop=(j == 3))
                if g % 2 == 1:
                    # batched DVE evac: y_g = ps * R (R broadcast along d)
                    y3 = yt[p][:, gs:gs + GW].rearrange(
                        "p (b d) -> p b d", d=D)
                    ps3 = ps[:, :].rearrange("p (b d) -> p b d", d=D)
                    r3 = R[:, 4 * g:4 * g + 4].rearrange(
                        "p (b o) -> p b o", o=1)
                    r3b, ps3b = broadcast_tensor_aps(r3, ps3)
                    nc.vector.tensor_tensor(y3, ps3b, r3b, op=Alu.mult)
                else:
                    for j in range(4):
                        c = 4 * g + j
                        nc.scalar.activation(yt[p][:, c * D:(c + 1) * D],
                                             ps[:, j * D:(j + 1) * D],
                                             Act.Copy, scale=R[:, c:c + 1])
            if do_dma:
                if p == pairs - 1:
                    qw = fw // 4
                    for qi in range(4):
                        nc.scalar.dma_start(yd[p][:, qi * qw:(qi + 1) * qw],
                                            yt[p][:, qi * qw:(qi + 1) * qw])
                else:
                    nc.scalar.dma_start(yd[p], yt[p][:])

        for p in range(pairs + 1):
            if p < pairs:
                phase1a(p)
            if p >= 1:
                chunks(p - 1)
            if p < pairs:
                phase1b(p)

    nc.compile()
    return nc


def pack_inputs(x, n=N):
    """[pairs_total, n, D] f32 -> [pairs_total, 128, nch*D] fp16 permuted."""
    nch = n // CH
    m = x.shape[0]
    xp = x.reshape(m, nch, CH, D).transpose(0, 2, 1, 3)  # [m, 128, nch, D]
    return np.ascontiguousarray(xp.reshape(m, 128, nch * D).astype(np.float16))


def unpack_output(yp, n=N):
    """[pairs_total, 128, nch*D] fp16 -> [pairs_total, n, D] f32."""
    nch = n // CH
    m = yp.shape[0]
    yv = yp.astype(np.float32).reshape(m, 128, nch, D)
    yv = yv.transpose(0, 2, 1, 3).reshape(m, nch * CH, D)
    return np.ascontiguousarray(yv)


_cached = {}


def _get_nc():
    if "nc" not in _cached:
        _cached["nc"] = build_nc()
    return _cached["nc"]


def run_on_hw(q, k, v, trace=False):
    """q,k,v: np [B,H,N,D] f32 -> (y [B,H,N,D], exec_time_ns or None)."""
    from concourse.bass_utils import run_bass_kernel_spmd

    nc = _get_nc()
    qp = pack_inputs(np.asarray(q, np.float32).reshape(B * H, N, D))
    kp = pack_inputs(np.asarray(k, np.float32).reshape(B * H, N, D))
    vp = pack_inputs(np.asarray(v, np.float32).reshape(B * H, N, D))
    qkvp = np.ascontiguousarray(np.concatenate([qp, kp, vp], axis=2))
    in_maps = [
        {"qkv": qkvp[c * PAIRS:(c + 1) * PAIRS]}
        for c in range(NCORES)
    ]
    try:
        res = run_bass_kernel_spmd(nc, in_maps, list(range(NCORES)), trace=trace)
    except Exception:
        if not trace:
            raise
        import traceback
        traceback.print_exc()
        print("trace=True path failed; retrying without trace", file=sys.stderr)
        res = run_bass_kernel_spmd(nc, in_maps, list(range(NCORES)), trace=False)
    yp = np.concatenate([np.asarray(res.results[c]["y"]) for c in range(NCORES)],
                        axis=0)
    return unpack_output(yp).reshape(B, H, N, D), res.exec_time_ns


def kernel(q, k, v):
    y, _ = run_on_hw(q, k, v, trace=False)
    return y
